# revision 8
# baseline (speedup 1.0000x reference)
"""Trainium2 Bass kernel for a DiverseBeamSearch step (step>0 path).

Strategy (data-parallel over batch, 8 rows per NeuronCore):
  Phase A (per core, DVE): for each of 128 partitions = (row, beam, half),
    stream 4 chunks of 6288 fp32 lprobs and take per-chunk top-8 values +
    indices with the DVE MAX8 / MAX_INDEX instructions. Only these
    64 candidates per beam can matter: the diversity penalty only lowers
    values, so the exact per-group top-(k + #penalized) = top-14 is always
    contained in the per-chunk top-8 union (proven + validated vs oracle).
  Gather: bounce the [128, 64] candidate block via DRAM to a row-major
    [8 rows, 1024] layout.
  Epilogue (DVE, tiny): per group apply mask/score-bias, sequential
    diversity penalties against previously selected tokens (exact f32 op
    order), top-2 select via MAX8/MAX_INDEX, pairwise token-overlap update.
All f32 arithmetic replicates the reference op-for-op bit-exactly.
"""
import numpy as np

BSZ, BEAM, VOCAB = 64, 8, 50257
G, SUB, K = 4, 2, 2
NCORES = 8
RPC = BSZ // NCORES          # rows per core
VPAD = 50304                 # 8 * 6288
CH = 6288                    # chunk length (free dim of one max8 call)
NQ = 4                       # chunks per partition (partition holds half a beam)
HALF = 2 * NQ * CH // 2      # 25152 elems per (beam, half)
NCB = 64                     # candidates per beam (8 chunks * 8)
W = 2 * NCB                  # candidates per group (2 sub-beams)
TOTW = G * W                 # 512 epilogue columns
NEG = np.float32(-1.0e30)

_CACHE = {}


def _build(loop_n=None):
    """loop_n: if set, wrap the body in an on-device repeat loop (bench only)."""
    import contextlib
    import concourse.bacc as bacc
    import concourse.tile as tile
    import concourse.mybir as mybir

    f32 = mybir.dt.float32
    i32 = mybir.dt.int32
    u32 = mybir.dt.uint32
    op = mybir.AluOpType

    nc = bacc.Bacc("TRN2", target_bir_lowering=False, debug=False,
                   num_devices=NCORES)

    lp_in = nc.declare_dram_parameter("lp", [NQ, 128, CH], f32, isOutput=False)
    cb_in = nc.declare_dram_parameter("cb", [RPC, TOTW], f32, isOutput=False)
    iota_in = nc.declare_dram_parameter("iota", [RPC, W], f32, isOutput=False)
    mbm_in = nc.declare_dram_parameter("mbm", [RPC, TOTW], f32, isOutput=False)
    mbs_in = nc.declare_dram_parameter("mbs", [RPC, TOTW], f32, isOutput=False)
    mneg_in = nc.declare_dram_parameter("mneg", [RPC, 8], f32, isOutput=False)
    pen_in = nc.declare_dram_parameter("pen", [RPC, 16], f32, isOutput=False)
    go_in = nc.declare_dram_parameter("goin", [RPC, 16], f32, isOutput=False)
    mk44_in = nc.declare_dram_parameter("mk44", [RPC, 32], f32, isOutput=False)

    souts_o = nc.declare_dram_parameter("souts", [RPC, 8], f32, isOutput=True)
    touts_o = nc.declare_dram_parameter("touts", [RPC, 8], i32, isOutput=True)
    bouts_o = nc.declare_dram_parameter("bouts", [RPC, 8], i32, isOutput=True)
    ngo_o = nc.declare_dram_parameter("ngo", [RPC, 16], f32, isOutput=True)

    mvb = nc.dram_tensor("mvb", [128, 64], f32)

    with tile.TileContext(nc) as tc:
        with tc.tile_pool(name="stream", bufs=3) as spool, \
             tc.tile_pool(name="cand", bufs=1) as cpool, \
             tc.tile_pool(name="small", bufs=1) as kpool, \
             tc.tile_pool(name="work", bufs=2) as wpool, \
             (tc.For_i(0, loop_n, 1) if loop_n else contextlib.nullcontext()):

            # ---- phase A ----
            mvi = cpool.tile([128, 64], f32)
            mvi_u = mvi[:].bitcast(u32)
            for q in range(NQ):
                xt = spool.tile([128, CH], f32, tag="xt")
                nc.sync.dma_start(xt[:], lp_in[q])
                nc.vector.max(mvi[:, q * 8:(q + 1) * 8], xt[:])
                nc.vector.max_index(mvi_u[:, 32 + q * 8:32 + (q + 1) * 8],
                                    mvi[:, q * 8:(q + 1) * 8], xt[:])

            # ---- bounce gather to row-major ----
            nc.sync.dma_start(mvb[:], mvi[:])
            cvt = cpool.tile([RPC, 2 * TOTW], f32)
            src = mvb.rearrange("(r sub g h) (pl c) -> sub h pl r g c",
                                r=RPC, sub=2, g=G, h=2, pl=2, c=32)
            for pl in range(2):
                dst = cvt[:, pl * TOTW:(pl + 1) * TOTW].rearrange(
                    "r (g sub h c) -> sub h r g c", g=G, sub=2, h=2, c=32)
                for sub in range(2):
                    for h in range(2):
                        nc.sync.dma_start(dst[sub, h], src[sub, h, pl])

            # ---- constants ----
            cb = kpool.tile([RPC, TOTW], f32)
            iota = kpool.tile([RPC, W], f32)
            mbm = kpool.tile([RPC, TOTW], f32)
            mbs = kpool.tile([RPC, TOTW], f32)
            mneg = kpool.tile([RPC, 8], f32)
            pen = kpool.tile([RPC, 16], f32)
            goin = kpool.tile([RPC, 16], f32)
            mk44 = kpool.tile([RPC, 32], f32)
            nc.sync.dma_start(cb[:], cb_in[:])
            nc.sync.dma_start(iota[:], iota_in[:])
            nc.sync.dma_start(mbm[:], mbm_in[:])
            nc.sync.dma_start(mbs[:], mbs_in[:])
            nc.sync.dma_start(mneg[:], mneg_in[:])
            nc.sync.dma_start(pen[:], pen_in[:])
            nc.sync.dma_start(goin[:], go_in[:])
            nc.sync.dma_start(mk44[:], mk44_in[:])

            # ---- epilogue ----
            ctf = cpool.tile([RPC, TOTW], f32)
            nc.vector.tensor_copy(ctf[:], cvt[:, TOTW:2 * TOTW].bitcast(u32))
            nc.vector.tensor_add(ctf[:], ctf[:], cb[:])
            cv = cvt[:, 0:TOTW]
            nc.vector.tensor_mul(cv, cv, mbm[:])
            nc.vector.tensor_add(cv, cv, mbs[:])

            souts = kpool.tile([RPC, 8], f32)
            toutf = kpool.tile([RPC, 8], f32)
            boutf = kpool.tile([RPC, 8], f32)

            for g in range(G):
                cvg = cv[:, g * W:(g + 1) * W]
                ctg = ctf[:, g * W:(g + 1) * W]
                if g > 0:
                    div = wpool.tile([RPC, W], f32, tag="div")
                    first = True
                    for i in range(K):
                        for pg in range(g):
                            eqt = wpool.tile([RPC, W], f32, tag="eqt")
                            nc.vector.tensor_scalar(
                                eqt[:], ctg, toutf[:, i * 4 + pg:i * 4 + pg + 1],
                                None, op0=op.is_equal)
                            if first:
                                nc.vector.tensor_scalar(
                                    div[:], eqt[:],
                                    pen[:, g * 4 + pg:g * 4 + pg + 1],
                                    None, op0=op.mult)
                                first = False
                            else:
                                nc.vector.scalar_tensor_tensor(
                                    div[:], eqt[:],
                                    pen[:, g * 4 + pg:g * 4 + pg + 1],
                                    div[:], op0=op.mult, op1=op.add)
                    for sub in range(SUB):
                        sl = slice(sub * NCB, (sub + 1) * NCB)
                        bm = g + 4 * sub
                        nc.vector.scalar_tensor_tensor(
                            cvg[:, sl], div[:, sl], mneg[:, bm:bm + 1],
                            cvg[:, sl], op0=op.mult, op1=op.add)

                mx = wpool.tile([RPC, 8], f32, tag="mx")
                mi = wpool.tile([RPC, 8], u32, tag="mi")
                nc.vector.max(mx[:], cvg)
                nc.vector.max_index(mi[:], mx[:], cvg)
                cf = wpool.tile([RPC, 2], f32, tag="cf")
                nc.vector.tensor_copy(cf[:], mi[:, 0:2])
                for i in range(K):
                    col = i * 4 + g
                    eqi = wpool.tile([RPC, W], f32, tag="eqi")
                    nc.vector.tensor_scalar(eqi[:], iota[:], cf[:, i:i + 1],
                                            None, op0=op.is_equal)
                    prod = wpool.tile([RPC, W], f32, tag="prod")
                    nc.vector.tensor_mul(prod[:], eqi[:], ctg)
                    nc.vector.reduce_sum(toutf[:, col:col + 1], prod[:],
                                         axis=mybir.AxisListType.X)
                    nc.vector.tensor_copy(souts[:, col:col + 1], mx[:, i:i + 1])
                    nc.vector.tensor_scalar(boutf[:, col:col + 1], cf[:, i:i + 1],
                                            float(NCB), 4.0,
                                            op0=op.is_ge, op1=op.mult)
                    nc.vector.tensor_scalar(boutf[:, col:col + 1],
                                            boutf[:, col:col + 1],
                                            float(g), None, op0=op.add)

            # ---- overlap matrix ----
            e = kpool.tile([RPC, 32], f32)
            for i in range(K):
                for g1 in range(G):
                    nc.vector.tensor_scalar(
                        e[:, i * 16 + g1 * 4:i * 16 + (g1 + 1) * 4],
                        toutf[:, i * 4:(i + 1) * 4],
                        toutf[:, i * 4 + g1:i * 4 + g1 + 1],
                        None, op0=op.is_equal)
            nc.vector.tensor_mul(e[:], e[:], mk44[:])
            ov = kpool.tile([RPC, 16], f32)
            nc.vector.tensor_add(ov[:], e[:, 0:16], e[:, 16:32])
            nc.vector.tensor_add(ov[:], ov[:], goin[:])
            ngo_t = kpool.tile([RPC, 16], f32)
            nc.vector.tensor_scalar(ngo_t[:], ov[:], 0.5, None, op0=op.mult)

            # ---- int conversions + outputs ----
            touti = kpool.tile([RPC, 8], i32)
            bouti = kpool.tile([RPC, 8], i32)
            nc.vector.tensor_copy(touti[:], toutf[:])
            nc.vector.tensor_copy(bouti[:], boutf[:])
            nc.sync.dma_start(souts_o[:], souts[:])
            nc.sync.dma_start(touts_o[:], touti[:])
            nc.sync.dma_start(bouts_o[:], bouti[:])
            nc.sync.dma_start(ngo_o[:], ngo_t[:])

    nc.compile()
    return nc


def _prep_in_maps(lprobs, scores, group_overlap, mask, step):
    lprobs = np.ascontiguousarray(np.asarray(lprobs, dtype=np.float32))
    scores = np.asarray(scores, dtype=np.float32)
    go = np.asarray(group_overlap, dtype=np.float32)
    maskf = np.asarray(mask).astype(np.float32)
    sc = np.ascontiguousarray(scores[:, :, step - 1])  # (BSZ, BEAM)

    lp_pad = np.full((BSZ, BEAM, VPAD), NEG, dtype=np.float32)
    lp_pad[:, :, :VOCAB] = lprobs

    # device per-core stream layout: [q, p, f], p = r*16 + sub*8 + g*2 + h
    # beam b = g + 4*sub  ->  b*2 + h == sub*8 + g*2 + h
    arr = lp_pad.reshape(BSZ, BEAM, 2, NQ, CH)          # [B, b, h, q, f]
    # per core: rows r, partitions (r, b, h) -> transpose to [q, r, b, h, f]

    # epilogue column tables, col = g*W + sub*NCB + h*32 + q*8 + j
    colg = np.arange(TOTW) // W
    colr = np.arange(TOTW) % W
    colsub = colr // NCB
    colh = (colr % NCB) // 32
    colq = (colr % 32) // 8
    colbeam = colg + 4 * colsub                          # global beam per col
    colbase = ((colh * NQ + colq) * CH).astype(np.float32)

    iota = np.broadcast_to(np.arange(W, dtype=np.float32), (RPC, W)).copy()

    in_maps = []
    for c in range(NCORES):
        rows = slice(c * RPC, (c + 1) * RPC)
        lp_c = np.ascontiguousarray(
            arr[rows].transpose(3, 0, 1, 2, 4).reshape(NQ, 128, CH))
        m_c = maskf[rows]                                # (RPC, 8)
        sc_c = sc[rows]
        go_c = go[rows]                                  # (RPC, 4, 4)
        cb_c = np.broadcast_to(colbase, (RPC, TOTW)).copy()
        mbm_c = m_c[:, colbeam].astype(np.float32)
        mbs_c = sc_c[:, colbeam].astype(np.float32)
        mneg_c = (np.float32(-0.5) * m_c).astype(np.float32)
        pen_c = (np.float32(1.0) + go_c).reshape(RPC, 16).astype(np.float32)
        go_flat = go_c.reshape(RPC, 16).astype(np.float32)
        # mk44[r, i*16+g1*4+g2] = m[r, i*4+g2]
        mk44 = np.zeros((RPC, 32), dtype=np.float32)
        for i in range(K):
            for g1 in range(G):
                mk44[:, i * 16 + g1 * 4:i * 16 + (g1 + 1) * 4] = \
                    m_c[:, i * 4:(i + 1) * 4] * m_c[:, i * 4 + g1:i * 4 + g1 + 1]
        in_maps.append({
            "lp": lp_c, "cb": cb_c, "iota": iota, "mbm": mbm_c, "mbs": mbs_c,
            "mneg": mneg_c, "pen": pen_c, "goin": go_flat, "mk44": mk44,
        })
    return in_maps


def kernel(lprobs, scores, group_overlap, mask, original_batch_idxs, step,
           **_unused):
    from concourse.bass_utils import run_bass_kernel_spmd

    step = int(np.asarray(step))
    obi = np.asarray(original_batch_idxs)
    assert np.array_equal(obi, np.arange(BSZ)), "kernel assumes identity batch idxs"

    if "nc" not in _CACHE:
        _CACHE["nc"] = _build()
    nc = _CACHE["nc"]

    in_maps = _prep_in_maps(lprobs, scores, group_overlap, mask, step)
    res = run_bass_kernel_spmd(nc, in_maps, core_ids=list(range(NCORES)))

    scores_buf = np.concatenate([r["souts"] for r in res.results], axis=0)
    indices_buf = np.concatenate([r["touts"] for r in res.results], axis=0)
    beams_buf = np.concatenate([r["bouts"] for r in res.results], axis=0)
    ngo = np.concatenate([r["ngo"] for r in res.results], axis=0)
    new_group_overlap = ngo.reshape(BSZ, G, G)
    return (scores_buf.astype(np.float32),
            indices_buf.astype(np.int32),
            beams_buf.astype(np.int32),
            new_group_overlap.astype(np.float32))


# revision 12
# speedup vs baseline: 1.0385x; 1.0385x over previous
"""Trainium2 Bass kernel for a DiverseBeamSearch step (step>0 path).

Strategy (data-parallel over batch, 8 rows per NeuronCore):
  Phase A (DVE): partitions = (row, beam, half); stream 4 chunks of 6288
    fp32 lprobs per partition and take per-chunk top-8 values + indices
    with MAX8 / MAX_INDEX. The diversity penalty only lowers values, so
    the exact per-group top-(k + #penalized) <= top-14 is always inside
    the per-chunk top-8 union (proven + validated vs oracle).
  Per-chunk bounce via DRAM rearranges candidates to row-major [8, 1024].
  Epilogue (DVE, tiny): mask/score-bias, sequential diversity penalties
    (exact f32 op order), top-2 select per group, pairwise token overlap.
All f32 arithmetic replicates the reference op-for-op bit-exactly.
"""
import numpy as np

BSZ, BEAM, VOCAB = 64, 8, 50257
G, SUB, K = 4, 2, 2
NCORES = 8
RPC = BSZ // NCORES          # rows per core
VPAD = 50304                 # 8 * 6288
CH = 6288                    # chunk length (free dim of one max8 call)
NQ = 4                       # chunks per partition (partition = half a beam)
NCB = 64                     # candidates per beam
W = 2 * NCB                  # candidates per group
TOTW = G * W                 # 512 epilogue columns
NEG = np.float32(-1.0e30)

# ctab layout (merged constants, one [RPC, CTW] f32 input)
O_CB, O_MBM, O_MBS = 0, 512, 1024
O_IOTA, O_MNEG, O_PEN, O_GO, O_MK = 1536, 1664, 1672, 1688, 1704
CTW = 1736

_CACHE = {}


def _build(loop_n=None):
    """loop_n: if set, wrap the body in an on-device repeat loop (bench only)."""
    import contextlib
    import concourse.bacc as bacc
    import concourse.tile as tile
    import concourse.mybir as mybir

    f32 = mybir.dt.float32
    u32 = mybir.dt.uint32
    op = mybir.AluOpType

    nc = bacc.Bacc("TRN2", target_bir_lowering=False, debug=False,
                   num_devices=NCORES)

    lp_in = nc.declare_dram_parameter("lp", [NQ, 128, CH], f32, isOutput=False)
    ct_in = nc.declare_dram_parameter("ctab", [RPC, CTW], f32, isOutput=False)

    vall_o = nc.declare_dram_parameter("vall", [RPC, 32], f32, isOutput=True)
    miall_o = nc.declare_dram_parameter("miall", [RPC, 32], u32, isOutput=True)
    tall_o = nc.declare_dram_parameter("tall", [RPC, 8], f32, isOutput=True)
    ngo_o = nc.declare_dram_parameter("ngo", [RPC, 16], f32, isOutput=True)

    mvb = nc.dram_tensor("mvb", [128, 64], f32)

    with tile.TileContext(nc) as tc:
        with tc.tile_pool(name="stream", bufs=4) as spool, \
             tc.tile_pool(name="cand", bufs=1) as cpool, \
             tc.tile_pool(name="small", bufs=1) as kpool, \
             tc.tile_pool(name="work", bufs=2) as wpool, \
             (tc.For_i(0, loop_n, 1) if loop_n else contextlib.nullcontext()):

            ctab = kpool.tile([RPC, CTW], f32)
            nc.sync.dma_start(ctab[:], ct_in[:])
            cvt = cpool.tile([RPC, 2 * TOTW], f32)
            # dst views for the gather: col = pl*512 + s*32 + q*8 + j,
            # s = (sub, g, h) 16 combos matching partition-minor order
            gdst = cvt[:].rearrange("r (pl s q j) -> pl q r s j",
                                    pl=2, s=16, q=NQ, j=8)
            gsrc = mvb.rearrange("(r m) (q pl j) -> q pl r m j",
                                 r=RPC, m=16, q=NQ, pl=2, j=8)

            # ---- phase A: per-chunk top-8 + pipelined bounce/gather ----
            for q in range(NQ):
                xt = spool.tile([128, CH], f32, tag="xt")
                nc.sync.dma_start(xt[:, 0:CH // 2], lp_in[q][:, 0:CH // 2])
                nc.scalar.dma_start(xt[:, CH // 2:CH], lp_in[q][:, CH // 2:CH])
                mviq = spool.tile([128, 16], f32, tag="mviq")
                nc.vector.max(mviq[:, 0:8], xt[:])
                nc.vector.max_index(mviq[:].bitcast(u32)[:, 8:16],
                                    mviq[:, 0:8], xt[:])
                nc.sync.dma_start(mvb[:, q * 16:(q + 1) * 16], mviq[:])
                nc.sync.dma_start(gdst[0, q], gsrc[q, 0])
                nc.sync.dma_start(gdst[1, q], gsrc[q, 1])

            # ---- epilogue ----
            ctf = cpool.tile([RPC, TOTW], f32)
            nc.vector.tensor_copy(ctf[:], cvt[:, TOTW:2 * TOTW].bitcast(u32))
            nc.vector.tensor_add(ctf[:], ctf[:], ctab[:, O_CB:O_CB + 512])
            cv = cvt[:, 0:TOTW]
            nc.vector.tensor_mul(cv, cv, ctab[:, O_MBM:O_MBM + 512])
            nc.vector.tensor_add(cv, cv, ctab[:, O_MBS:O_MBS + 512])
            cv3 = cvt[:, 0:TOTW].rearrange("r (sub x) -> r sub x", sub=2)
            ctf3 = ctf[:].rearrange("r (sub x) -> r sub x", sub=2)
            iota3 = ctab[:, O_IOTA:O_IOTA + 128].rearrange(
                "r (sub x) -> r sub x", sub=2)

            vall = kpool.tile([RPC, 32], f32)
            miall = kpool.tile([RPC, 32], u32)
            tall = kpool.tile([RPC, 8], f32)   # tokens, col = g*2 + i

            for g in range(G):
                cvg = cv3[:, :, g * NCB:(g + 1) * NCB]
                ctg = ctf3[:, :, g * NCB:(g + 1) * NCB]
                if g > 0:
                    div = wpool.tile([RPC, W], f32, tag="div")
                    div3 = div[:].rearrange("r (sub x) -> r sub x", sub=2)
                    for pg in range(g):
                        eqt = wpool.tile([RPC, W], f32, tag="eqt")
                        eqt3 = eqt[:].rearrange("r (sub x) -> r sub x", sub=2)
                        nc.vector.tensor_scalar(
                            eqt3, ctg, tall[:, pg * 2:pg * 2 + 1],
                            None, op0=op.is_equal)
                        nc.vector.scalar_tensor_tensor(
                            eqt3, ctg, tall[:, pg * 2 + 1:pg * 2 + 2],
                            eqt3, op0=op.is_equal, op1=op.add)
                        if pg == 0:
                            nc.vector.tensor_scalar(
                                div[:], eqt[:],
                                ctab[:, O_PEN + g * 4:O_PEN + g * 4 + 1],
                                None, op0=op.mult)
                        else:
                            nc.vector.scalar_tensor_tensor(
                                div[:], eqt[:],
                                ctab[:, O_PEN + g * 4 + pg:O_PEN + g * 4 + pg + 1],
                                div[:], op0=op.mult, op1=op.add)
                    for sub in range(SUB):
                        bm = g + 4 * sub
                        nc.vector.scalar_tensor_tensor(
                            cvg[:, sub, :], div3[:, sub, :],
                            ctab[:, O_MNEG + bm:O_MNEG + bm + 1],
                            cvg[:, sub, :], op0=op.mult, op1=op.add)

                cvgc = wpool.tile([RPC, W], f32, tag="cvgc")
                nc.vector.tensor_copy(
                    cvgc[:].rearrange("r (sub x) -> r sub x", sub=2), cvg)
                nc.vector.max(vall[:, g * 8:(g + 1) * 8], cvgc[:])
                nc.vector.max_index(miall[:, g * 8:(g + 1) * 8],
                                    vall[:, g * 8:(g + 1) * 8], cvgc[:])
                cf = wpool.tile([RPC, 2], f32, tag="cf")
                nc.vector.tensor_copy(cf[:], miall[:, g * 8:g * 8 + 2])
                for i in range(K):
                    prod = wpool.tile([RPC, W], f32, tag="prod")
                    prod3 = prod[:].rearrange("r (sub x) -> r sub x", sub=2)
                    nc.vector.scalar_tensor_tensor(
                        prod3, iota3, cf[:, i:i + 1], ctg,
                        op0=op.is_equal, op1=op.mult)
                    nc.vector.reduce_sum(tall[:, g * 2 + i:g * 2 + i + 1],
                                         prod[:], axis=mybir.AxisListType.X)

            # ---- overlap matrix ----
            e = kpool.tile([RPC, 32], f32)
            for i in range(K):
                for g1 in range(G):
                    nc.vector.tensor_scalar(
                        e[:, i * 16 + g1 * 4:i * 16 + (g1 + 1) * 4],
                        tall[:, i::2],
                        tall[:, g1 * 2 + i:g1 * 2 + i + 1],
                        None, op0=op.is_equal)
            nc.vector.tensor_mul(e[:], e[:], ctab[:, O_MK:O_MK + 32])
            ov = kpool.tile([RPC, 16], f32)
            nc.vector.tensor_add(ov[:], e[:, 0:16], e[:, 16:32])
            nc.vector.tensor_add(ov[:], ov[:], ctab[:, O_GO:O_GO + 16])
            ngo_t = kpool.tile([RPC, 16], f32)
            nc.vector.tensor_scalar(ngo_t[:], ov[:], 0.5, None, op0=op.mult)

            nc.sync.dma_start(vall_o[:], vall[:])
            nc.sync.dma_start(miall_o[:], miall[:])
            nc.sync.dma_start(tall_o[:], tall[:])
            nc.sync.dma_start(ngo_o[:], ngo_t[:])

    nc.compile()
    return nc


def _prep_in_maps(lprobs, scores, group_overlap, mask, step):
    lprobs = np.ascontiguousarray(np.asarray(lprobs, dtype=np.float32))
    scores = np.asarray(scores, dtype=np.float32)
    go = np.asarray(group_overlap, dtype=np.float32)
    maskf = np.asarray(mask).astype(np.float32)
    sc = np.ascontiguousarray(scores[:, :, step - 1])  # (BSZ, BEAM)

    lp_pad = np.full((BSZ, BEAM, VPAD), NEG, dtype=np.float32)
    lp_pad[:, :, :VOCAB] = lprobs
    # partition p = r*16 + sub*8 + g*2 + h  (beam = g + 4*sub)
    arr = lp_pad.reshape(BSZ, BEAM, 2, NQ, CH)          # [B, beam, h, q, f]

    # epilogue col (within 512) = sub*256 + g*64 + h*32 + q*8 + j
    col = np.arange(TOTW)
    colsub = col // 256
    colg = (col % 256) // 64
    colh = (col % 64) // 32
    colq = (col % 32) // 8
    colbeam = colg + 4 * colsub
    colbase = ((colh * NQ + colq) * CH).astype(np.float32)

    in_maps = []
    for c in range(NCORES):
        rows = slice(c * RPC, (c + 1) * RPC)
        m_c = maskf[rows]
        sc_c = sc[rows]
        go_c = go[rows]
        ctab = np.zeros((RPC, CTW), dtype=np.float32)
        ctab[:, O_CB:O_CB + 512] = colbase
        ctab[:, O_MBM:O_MBM + 512] = m_c[:, colbeam]
        ctab[:, O_MBS:O_MBS + 512] = sc_c[:, colbeam]
        ctab[:, O_IOTA:O_IOTA + 128] = np.arange(W, dtype=np.float32)
        ctab[:, O_MNEG:O_MNEG + 8] = np.float32(-0.5) * m_c
        ctab[:, O_PEN:O_PEN + 16] = (np.float32(1.0) + go_c).reshape(RPC, 16)
        ctab[:, O_GO:O_GO + 16] = go_c.reshape(RPC, 16)
        for i in range(K):
            for g1 in range(G):
                ctab[:, O_MK + i * 16 + g1 * 4:O_MK + i * 16 + (g1 + 1) * 4] = \
                    m_c[:, i * 4:(i + 1) * 4] * m_c[:, i * 4 + g1:i * 4 + g1 + 1]
        # lp layout [q, p, f]: p = r*16 + sub*8 + g*2 + h; beam = sub*4 + g
        b = arr[rows].reshape(RPC, SUB, G, 2, NQ, CH)    # [r, sub, g, h, q, f]
        lp_c = np.ascontiguousarray(
            b.transpose(4, 0, 1, 2, 3, 5).reshape(NQ, 128, CH))
        in_maps.append({"lp": lp_c, "ctab": ctab})
    return in_maps


def kernel(lprobs, scores, group_overlap, mask, original_batch_idxs, step,
           **_unused):
    from concourse.bass_utils import run_bass_kernel_spmd

    step = int(np.asarray(step))
    obi = np.asarray(original_batch_idxs)
    assert np.array_equal(obi, np.arange(BSZ)), "kernel assumes identity batch idxs"

    if "nc" not in _CACHE:
        _CACHE["nc"] = _build()
    nc = _CACHE["nc"]

    in_maps = _prep_in_maps(lprobs, scores, group_overlap, mask, step)
    res = run_bass_kernel_spmd(nc, in_maps, core_ids=list(range(NCORES)))

    vall = np.concatenate([r["vall"] for r in res.results], axis=0)   # (B,32)
    miall = np.concatenate([r["miall"] for r in res.results], axis=0)
    tall = np.concatenate([r["tall"] for r in res.results], axis=0)   # (B,8)
    ngo = np.concatenate([r["ngo"] for r in res.results], axis=0)

    scores_buf = np.zeros((BSZ, 8), dtype=np.float32)
    indices_buf = np.zeros((BSZ, 8), dtype=np.int32)
    beams_buf = np.zeros((BSZ, 8), dtype=np.int32)
    for i in range(K):
        for g in range(G):
            scores_buf[:, i * G + g] = vall[:, g * 8 + i]
            indices_buf[:, i * G + g] = tall[:, g * 2 + i].astype(np.int32)
            c = miall[:, g * 8 + i].astype(np.int64)
            beams_buf[:, i * G + g] = (c // NCB) * 4 + g
    new_group_overlap = ngo.reshape(BSZ, G, G).astype(np.float32)
    return (scores_buf, indices_buf, beams_buf, new_group_overlap)
